# revision 7
# baseline (speedup 1.0000x reference)
"""Trainium2 Bass kernel for sparse-input LIF layer (KerasLIFLayerSparse).

Contract: kernel(**inputs) takes FULL unsharded inputs (as produced by
setup_inputs) and returns the full (out_spike_ids, num_out_spikes, states)
tuple, matching reference().

Strategy (8 NeuronCores, data-parallel over batch, 4 batch rows per core):
  - Host: build dense one-hot COUNTS from spike ids (input re-encoding),
    transpose W, shard over batch.
  - Device phase B (TensorE): SYN = W'^T-matmul over the one-hot counts,
    time-blocked (8 blocks x 128 steps), PSUM-accumulated over the 1024
    contraction dim; ScalarE evacuates PSUM with the (1-decay) scale.
  - Device phase C (VectorE): sequential 1024-step LIF scan; per step two
    fused scalar_tensor_tensor ops on [128, 32] tiles:
        v = state*decay + syn ; state = (v <= thr) * v
  - Device phase D: TensorE transposes v into [timebatch, unit] tiles;
    VectorE builds val = (v > thr)*(1024-u) and extracts the first 32
    spiking unit ids (ascending) exactly via 4 rounds of max8+match_replace;
    spike counts via tensor_scalar accumulate.
  - Host: undo layout shuffles, trim time padding, concat batch shards.
"""

import os
import sys

try:
    import concourse  # noqa: F401
except ImportError:
    sys.path.insert(0, "/opt/trn_rl_repo")

import numpy as np


def _ensure_ntff_hook():
    """Provide antenv.axon_hooks (missing on this image) so trace=True works."""
    try:
        from antenv.axon_hooks import get_axon_ntff_profile_hook  # noqa: F401
        return
    except ImportError:
        pass
    import types

    mod = types.ModuleType("antenv.axon_hooks")
    _hook = [None]
    mod.set_axon_ntff_profile_hook = lambda h: _hook.__setitem__(0, h)
    mod.get_axon_ntff_profile_hook = lambda: _hook[0]
    sys.modules["antenv.axon_hooks"] = mod
    try:
        import antenv

        antenv.axon_hooks = mod
    except ImportError:
        pass
    try:
        from trn_agent_boot.trn_boot import _ntff_profile_via_ctypes

        mod.set_axon_ntff_profile_hook(
            _ntff_profile_via_ctypes("/opt/axon/libaxon_pjrt.so")
        )
    except Exception:
        pass

import concourse.bacc as bacc
import concourse.mybir as mybir
import concourse.tile as tile
from concourse.bass_utils import run_bass_kernel_spmd
from concourse.masks import make_identity

# Problem constants (hardcoded per harness contract)
T = 1000
B = 32
U = 1024
IN = 1024
K_IN = 32
SSO = 32  # size_sparse_out

NCORES = 8
BL = B // NCORES          # batch rows per core (4)
T_PAD = 1024
NB = 8                    # time blocks
L = T_PAD // NB           # steps per block (128)
NCOL = 32                 # scan-tile columns: 8 uchunks x 4 batch
UC = 8                    # unit chunks of 128
IC = 8                    # input-dim chunks of 128
NMM = L * BL              # matmul moving cols per block per uchunk (512)
NG = L // 32              # [128,1024]-row groups per block in phase D (4)

_CACHE = {}

F32 = mybir.dt.float32


def _build(decay: float, thr: float):
    nc = bacc.Bacc(None)
    one_minus_d = float(np.float32(1.0) - np.float32(decay))

    wt_d = nc.dram_tensor("wt", [128, IC * U], F32, kind="ExternalInput")
    cnts_d = nc.dram_tensor("cnts", [NB, 128, IC * NMM], F32, kind="ExternalInput")
    st0_d = nc.dram_tensor("st0", [128, NCOL], F32, kind="ExternalInput")
    vraw_d = nc.dram_tensor("vraw", [NB, 128, L * NCOL], F32, kind="ExternalOutput")
    oids_d = nc.dram_tensor("oids", [NB, NG, 128, SSO], F32, kind="ExternalOutput")
    nout_d = nc.dram_tensor("nout", [NB, NG, 128, 1], F32, kind="ExternalOutput")

    with tile.TileContext(nc) as tc:
        with (
            tc.tile_pool(name="const", bufs=1) as const_pool,
            tc.tile_pool(name="cnts", bufs=2) as cnts_pool,
            tc.tile_pool(name="syn", bufs=2) as syn_pool,
            tc.tile_pool(name="vbuf", bufs=2) as v_pool,
            tc.tile_pool(name="psB", bufs=3, space="PSUM") as psB,
            tc.tile_pool(name="psD", bufs=2, space="PSUM") as psD,
            tc.tile_pool(name="vtb", bufs=2) as vtb_pool,
            tc.tile_pool(name="val", bufs=2) as val_pool,
            tc.tile_pool(name="small", bufs=4) as small_pool,
        ):
            # --- constants ---
            wt_sb = const_pool.tile([128, IC * U], F32, tag="wt")
            nc.sync.dma_start(wt_sb[:], wt_d[:])
            state = const_pool.tile([128, NCOL], F32, tag="state")
            nc.sync.dma_start(state[:], st0_d[:])
            ident = const_pool.tile([128, 128], F32, tag="ident")
            make_identity(nc, ident[:])
            iota_i = const_pool.tile([128, U], mybir.dt.int32, tag="iotai")
            nc.gpsimd.iota(iota_i[:], pattern=[[-1, U]], base=U, channel_multiplier=0)
            iota_desc = const_pool.tile([128, U], F32, tag="iotaf")
            nc.vector.tensor_copy(iota_desc[:], iota_i[:])

            for nb in range(NB):
                # ---- phase B: SYN(block) = (1-d) * W^T @ counts ----
                cnts_sb = cnts_pool.tile([128, IC * NMM], F32)
                nc.sync.dma_start(cnts_sb[:], cnts_d[nb])
                # syn/vblk layout: col = uc*NMM + t*BL + b
                syn = syn_pool.tile([128, L * NCOL], F32)
                for uc in range(UC):
                    ps = psB.tile([128, NMM], F32)
                    for ic in range(IC):
                        nc.tensor.matmul(
                            ps[:],
                            lhsT=wt_sb[:, ic * U + uc * 128 : ic * U + (uc + 1) * 128],
                            rhs=cnts_sb[:, ic * NMM : (ic + 1) * NMM],
                            start=(ic == 0),
                            stop=(ic == IC - 1),
                        )
                    # evacuate with (1-d) scale (contiguous dest)
                    nc.scalar.mul(
                        syn[:, uc * NMM : (uc + 1) * NMM], ps[:], one_minus_d
                    )

                # ---- phase C: sequential scan over L steps ----
                vblk = v_pool.tile([128, L * NCOL], F32)
                syn_4 = syn[:].rearrange("p (u t b) -> p u t b", u=UC, b=BL)
                vblk_4 = vblk[:].rearrange("p (u t b) -> p u t b", u=UC, b=BL)
                state_3 = state[:].rearrange("p (u b) -> p u b", b=BL)
                for t in range(L):
                    nc.vector.scalar_tensor_tensor(
                        out=vblk_4[:, :, t, :],
                        in0=state_3,
                        scalar=decay,
                        in1=syn_4[:, :, t, :],
                        op0=mybir.AluOpType.mult,
                        op1=mybir.AluOpType.add,
                    )
                    nc.vector.scalar_tensor_tensor(
                        out=state_3,
                        in0=vblk_4[:, :, t, :],
                        scalar=thr,
                        op0=mybir.AluOpType.is_le,
                        op1=mybir.AluOpType.mult,
                        in1=vblk_4[:, :, t, :],
                    )
                nc.sync.dma_start(vraw_d[nb], vblk[:])

                # ---- phase D: first-32 spiking ids per (t, b) row ----
                for g in range(NG):
                    vtb = vtb_pool.tile([128, U], F32)
                    for uc in range(UC):
                        pst = psD.tile([128, 128], F32)
                        src = vblk[:, uc * NMM + g * 128 : uc * NMM + (g + 1) * 128]
                        nc.tensor.transpose(pst[:], src, ident[:])
                        nc.scalar.copy(vtb[:, uc * 128 : (uc + 1) * 128], pst[:])
                    # val = (v > thr) * (U - u)  [fp32 ints, exact]
                    val = val_pool.tile([128, U], F32, tag="val")
                    nc.vector.scalar_tensor_tensor(
                        out=val[:],
                        in0=vtb[:],
                        scalar=thr,
                        op0=mybir.AluOpType.is_gt,
                        op1=mybir.AluOpType.mult,
                        in1=iota_desc[:],
                    )
                    # spike count via accumulate
                    ind = val_pool.tile([128, U], F32, tag="ind")
                    cnt = small_pool.tile([128, 1], F32, tag="cnt")
                    nc.vector.tensor_scalar(
                        out=ind[:],
                        in0=vtb[:],
                        scalar1=thr,
                        scalar2=1.0,
                        op0=mybir.AluOpType.is_gt,
                        op1=mybir.AluOpType.mult,
                        accum_out=cnt[:],
                    )
                    # 4 rounds of top-8 extraction (descending val = ascending id)
                    mall = small_pool.tile([128, SSO], F32, tag="mall")
                    for r in range(SSO // 8):
                        nc.vector.max(out=mall[:, r * 8 : (r + 1) * 8], in_=val[:])
                        if r < SSO // 8 - 1:
                            nc.vector.match_replace(
                                out=val[:],
                                in_to_replace=mall[:, r * 8 : (r + 1) * 8],
                                in_values=val[:],
                                imm_value=0.0,
                            )
                    # decode: ids = (val>0) * (U - val) ; num = min(cnt, 32)
                    ids_raw = small_pool.tile([128, SSO], F32, tag="idsraw")
                    nc.vector.tensor_scalar(
                        out=ids_raw[:],
                        in0=mall[:],
                        scalar1=float(U),
                        scalar2=-1.0,
                        op0=mybir.AluOpType.subtract,
                        op1=mybir.AluOpType.mult,
                    )
                    oid = small_pool.tile([128, SSO], F32, tag="oid")
                    nc.vector.scalar_tensor_tensor(
                        out=oid[:],
                        in0=mall[:],
                        scalar=0.0,
                        op0=mybir.AluOpType.is_gt,
                        op1=mybir.AluOpType.mult,
                        in1=ids_raw[:],
                    )
                    nout_t = small_pool.tile([128, 1], F32, tag="noutt")
                    nc.vector.tensor_scalar(
                        out=nout_t[:],
                        in0=cnt[:],
                        scalar1=float(SSO),
                        scalar2=None,
                        op0=mybir.AluOpType.min,
                    )
                    nc.sync.dma_start(oids_d[nb, g], oid[:])
                    nc.sync.dma_start(nout_d[nb, g], nout_t[:])

    nc.finalize()
    return nc


def _host_prep(w, init_state, inp_spike_ids, num_inp_spikes):
    """Build per-core input maps."""
    ids = inp_spike_ids.astype(np.int32)  # [T, B, K]
    num = num_inp_spikes.astype(np.int32)  # [T, B, 1]
    valid = np.arange(K_IN, dtype=np.int32)[None, None, :] < num  # [T, B, K]

    # dense one-hot counts [T, B, IN] fp32
    counts = np.zeros((T * B, IN), dtype=np.float32)
    row = np.repeat(np.arange(T * B, dtype=np.int64)[:, None], K_IN, axis=1)
    np.add.at(counts, (row[valid.reshape(T * B, K_IN)],
                       ids.reshape(T * B, K_IN)[valid.reshape(T * B, K_IN)]), 1.0)
    counts = counts.reshape(T, B, IN)
    # pad time to T_PAD
    cpad = np.zeros((T_PAD, B, IN), dtype=np.float32)
    cpad[:T] = counts

    # wT laid as [128, ic*1024 + u]
    wt = np.ascontiguousarray(
        w.T.reshape(IC, 128, U).transpose(1, 0, 2).reshape(128, IC * U)
    ).astype(np.float32)

    in_maps = []
    for c in range(NCORES):
        cc = cpad[:, c * BL : (c + 1) * BL, :]  # [T_PAD, BL, IN]
        # target [NB, 128(i128), IC, L, BL] -> [NB, 128, IC*NMM]
        ch = (
            cc.reshape(NB, L, BL, IC, 128)
            .transpose(0, 4, 3, 1, 2)
            .reshape(NB, 128, IC * NMM)
        )
        st = init_state[c * BL : (c + 1) * BL, :]  # [BL, U]
        st0 = (
            st.reshape(BL, UC, 128).transpose(2, 1, 0).reshape(128, NCOL)
        )
        in_maps.append(
            {
                "wt": wt,
                "cnts": np.ascontiguousarray(ch),
                "st0": np.ascontiguousarray(st0.astype(np.float32)),
            }
        )
    return in_maps


def _host_post(results):
    out_ids = np.empty((T, B, SSO), dtype=np.float32)
    n_out = np.empty((T, B, 1), dtype=np.float32)
    states = np.empty((T, B, U), dtype=np.float32)
    for c, res in enumerate(results):
        bsl = slice(c * BL, (c + 1) * BL)
        # vraw [NB, 128(u128), UC, L, BL] -> [T_PAD, BL, U]
        v = (
            res["vraw"]
            .reshape(NB, 128, UC, L, BL)
            .transpose(0, 3, 4, 2, 1)
            .reshape(T_PAD, BL, U)
        )
        states[:, bsl, :] = v[:T]
        # oids [NB, NG, 128(t32*BL), SSO] -> [T_PAD, BL, SSO]
        o = res["oids"].reshape(NB, NG, 32, BL, SSO).reshape(T_PAD, BL, SSO)
        out_ids[:, bsl, :] = o[:T]
        n = res["nout"].reshape(NB, NG, 32, BL, 1).reshape(T_PAD, BL, 1)
        n_out[:, bsl, :] = n[:T]
    return out_ids, n_out, states


def kernel(w, init_state, inp_spike_ids, num_inp_spikes, decay_constants,
           thresholds):
    w = np.asarray(w, dtype=np.float32)
    init_state = np.asarray(init_state, dtype=np.float32)
    inp_spike_ids = np.asarray(inp_spike_ids)
    num_inp_spikes = np.asarray(num_inp_spikes)
    decay = float(np.asarray(decay_constants).ravel()[0])
    thr = float(np.asarray(thresholds).ravel()[0])

    key = (decay, thr)
    if key not in _CACHE:
        _CACHE[key] = _build(decay, thr)
    nc = _CACHE[key]

    in_maps = _host_prep(w, init_state, inp_spike_ids, num_inp_spikes)
    trace = bool(int(os.environ.get("LIF_TRACE", "0")))
    if trace:
        _ensure_ntff_hook()
    res = run_bass_kernel_spmd(
        nc, in_maps, core_ids=list(range(NCORES)), trace=trace
    )
    global LAST_RESULT
    LAST_RESULT = res
    return _host_post(res.results)


LAST_RESULT = None
